# revision 11
# baseline (speedup 1.0000x reference)
"""GateRetention Trainium2 kernel (Bass/Tile), 8-core tensor-parallel.

Sharding: core grid (batch b = core//4, head-group g = core%4); each core owns
4 heads (512 cols of the q/k/v/g projections, 512 rows of Wo) of one batch.
RMS-norm statistics are AllReduced across each batch's 4 cores; out-proj
partials are summed on the host (row-parallel TP gather).

kernel(**inputs) takes the FULL inputs from reference.setup_inputs() and
returns the FULL [B, T, DIM] output.
"""
import os
import sys

sys.path.insert(0, "/opt/trn_rl_repo")

import numpy as np

import concourse.bass as bass
import concourse.bacc as bacc
import concourse.tile as tile
import concourse.mybir as mybir
from concourse import bass_utils

F32 = mybir.dt.float32
F32R = mybir.dt.float32r
AX = mybir.AxisListType
ALU = mybir.AluOpType
ACTF = mybir.ActivationFunctionType

B, T, DIM = 2, 4096, 2048
H, HD = 16, 128
CS = 256
NCH = T // CS              # 16 chunks
EPS = 1e-5
GLN = 16.0
SCALE = HD ** -0.5
NCORE = 8
HPC = 4                    # heads per core
PCOLS = HPC * HD           # 512 cols per core
NBLK = T // 128            # 32 token blocks of 128

DEBUG = bool(int(os.environ.get("GR_DEBUG", "0")))
TRACE = bool(int(os.environ.get("GR_TRACE", "0")))

_cache = {}


def _consts_np():
    """[128, 512] fp32: identity | Lm | Om | Umask."""
    ident = np.eye(128, dtype=np.float32)
    # cumsum matrices (scaled by -1/GLN since the input is softplus(-z)):
    # b[i] = sum_{j<=i} -sp[j]/GLN  ->  Lm[j, i] = -1/GLN for j <= i
    jj, ii = np.meshgrid(np.arange(128), np.arange(128), indexing="ij")
    Lm = np.where(jj <= ii, -1.0 / GLN, 0.0).astype(np.float32)
    Om = np.full((128, 128), -1.0 / GLN, np.float32)
    # AT diagonal mask: keep ci >= cj  (rows=cj, cols=ci)
    Um = np.where(jj <= ii, 1.0, 0.0).astype(np.float32)
    ones = np.ones((128, 8), np.float32)
    return np.concatenate([ident, Lm, Om, Um, ones], axis=1)


def build(debug=False):
    nc = bacc.Bacc("TRN2", target_bir_lowering=False, debug=False,
                   enable_asserts=False, num_devices=NCORE)

    # ---------------- I/O ----------------
    xT = nc.dram_tensor("xT", [DIM, T], F32R, kind="ExternalInput").ap()
    cT = nc.dram_tensor("cT", [DIM, T], F32R, kind="ExternalInput").ap()
    wq = nc.dram_tensor("wq", [DIM, PCOLS], F32R, kind="ExternalInput").ap()
    wk = nc.dram_tensor("wk", [DIM, PCOLS], F32R, kind="ExternalInput").ap()
    wv = nc.dram_tensor("wv", [DIM, PCOLS], F32R, kind="ExternalInput").ap()
    wg = nc.dram_tensor("wg", [DIM, PCOLS], F32R, kind="ExternalInput").ap()
    wgt = nc.dram_tensor("wgt", [DIM, HPC], F32R, kind="ExternalInput").ap()
    wo = nc.dram_tensor("wo", [PCOLS, DIM], F32R, kind="ExternalInput").ap()
    consts = nc.dram_tensor("consts", [128, 520], F32R, kind="ExternalInput").ap()
    out = nc.dram_tensor("out", [T, DIM], F32, kind="ExternalOutput").ap()

    def dbg(name, shape, dtype=F32):
        return nc.dram_tensor(name, shape, dtype, kind="ExternalOutput").ap()

    with tile.TileContext(nc) as tc:
        with (
            tc.tile_pool(name="const", bufs=1) as cpool,
            tc.tile_pool(name="wts", bufs=1) as wpool,
            tc.tile_pool(name="xstream", bufs=2) as xpool,
            tc.tile_pool(name="cstream", bufs=2) as ctpool,
            tc.tile_pool(name="evac", bufs=2) as epool,
            tc.tile_pool(name="persist", bufs=1) as ppool,
            tc.tile_pool(name="small", bufs=2) as spool,
            tc.tile_pool(name="ret", bufs=2) as rpool,
            tc.tile_pool(name="ps", bufs=1, space="PSUM") as psp,
            tc.tile_pool(name="dram", bufs=1, space="DRAM") as dpool,
        ):
            def ps_big():
                return psp.tile([128, 512], F32, tag="big", bufs=4, name="psbig")

            def ps_small(shape=None, dtype=F32):
                return psp.tile(shape or [128, 256], dtype, tag="small", bufs=4, name="pssmall")

            # ---------------- constants ----------------
            cst = cpool.tile([128, 520], F32R, tag="consts")
            nc.sync.dma_start(cst[:], consts)
            ident = cst[:, 0:128]
            ident32 = ident.bitcast(F32)
            Lm = cst[:, 128:256]
            Om = cst[:, 256:384]
            Um = cst[:, 384:512]
            Um32 = Um.bitcast(F32)

            ones1 = cst[:, 512:513]

            # ---------------- DRAM scratch ----------------
            if debug:
                qT_s = dbg("dbg_qT", [PCOLS, T], F32R)
                kT_s = dbg("dbg_kT", [PCOLS, T], F32R)
                vN_s = dbg("dbg_vN", [T, PCOLS], F32R)
                gT_s = dbg("dbg_gT", [PCOLS, T], F32R)
            else:
                qT_s = dpool.tile([PCOLS, T], F32R)
                kT_s = dpool.tile([PCOLS, T], F32R)
                vN_s = dpool.tile([T, PCOLS], F32R)
                gT_s = dpool.tile([PCOLS, T], F32R)
            ss_in = dpool.tile([3, T], F32)
            ss_out = dpool.tile([3, T], F32)

            # =========================================================
            # P1: projections (two passes), fp32r
            #   pass A: qT, kT (T-layout, lhsT=W)
            #   pass B: v natural (lhsT=xT), gT (T-layout), gt logits
            # =========================================================
            NT = T // 512  # 8 token n-tiles

            gtn = ppool.tile([128, NBLK, HPC], F32, tag="gtn")
            vss = ppool.tile([128, NBLK], F32, tag="vss")

            def load_w(wdram, tag):
                wt = wpool.tile([128, 16, 512], F32R, tag=tag)
                nc.sync.dma_start(
                    wt[:], wdram.rearrange("(kt p) m -> p kt m", p=128))
                return wt

            def xt_halves(n):
                tok = slice(n * 512, (n + 1) * 512)
                halves = []
                for h2 in range(2):
                    xt = xpool.tile([128, 8, 512], F32R, tag="xt")
                    nc.sync.dma_start(
                        xt[:], xT[h2 * 1024:(h2 + 1) * 1024, tok].rearrange(
                            "(kt p) m -> p kt m", p=128))
                    halves.append(xt)
                return halves

            def tproj_mms(ps, wt, xth, m):
                for k in range(16):
                    nc.tensor.matmul(
                        ps[:], wt[:, k, m * 128:(m + 1) * 128],
                        xth[k // 8][:, k % 8, :], start=(k == 0), stop=(k == 15))

            # ---------- pass A: q, k ----------
            wts_a = [load_w(wq, "w0"), load_w(wk, "w1")]
            for n in range(NT):
                tok = slice(n * 512, (n + 1) * 512)
                xth = xt_halves(n)
                for pi, sdram in enumerate((qT_s, kT_s)):
                    for m in range(4):
                        ps = ps_big()
                        tproj_mms(ps, wts_a[pi], xth, m)
                        ev = epool.tile([128, 512], F32R, tag="ev")
                        if m % 2 == 0:
                            nc.vector.tensor_copy(ev[:], ps[:])
                        else:
                            nc.scalar.copy(ev[:], ps[:])
                        nc.sync.dma_start(sdram[m * 128:(m + 1) * 128, tok], ev[:])
                        sqt = epool.tile([128, 512], F32R, tag="sq")
                        nc.scalar.activation(sqt[:], ev[:], ACTF.Square)
                        if m == 0:
                            ssps = ps_small([1, 512])
                        nc.tensor.matmul(ssps[:1, :], ones1[:], sqt[:],
                                         start=(m == 0), stop=(m == 3))
                        if m == 3:
                            ssev = spool.tile([1, 512], F32, tag="ssev", bufs=1)
                            nc.vector.tensor_copy(ssev[:], ssps[:1, :])
                            nc.sync.dma_start(ss_in[pi:pi + 1, tok], ssev[:])

            # ---------- pass B: v natural, g, gt ----------
            wv_sb = load_w(wv, "w0")
            wg_sb = load_w(wg, "w1")
            wgt_sb = wpool.tile([128, 16, HPC], F32R, tag="wgt")
            nc.sync.dma_start(wgt_sb[:],
                              wgt.rearrange("(kt p) m -> p kt m", p=128))
            for n in range(NT):
                tok = slice(n * 512, (n + 1) * 512)
                xth = xt_halves(n)
                # v natural: out rows = tokens
                for mt in range(4):
                    msl = slice(mt * 128, (mt + 1) * 128)
                    ps = ps_big()
                    for k in range(16):
                        nc.tensor.matmul(
                            ps[:], xth[k // 8][:, k % 8, msl], wv_sb[:, k, :],
                            start=(k == 0), stop=(k == 15))
                    ev = epool.tile([128, 512], F32R, tag="ev")
                    if mt % 2 == 0:
                        nc.vector.tensor_copy(ev[:], ps[:])
                    else:
                        nc.scalar.copy(ev[:], ps[:])
                    nc.sync.dma_start(vN_s[n * 512 + mt * 128:
                                           n * 512 + (mt + 1) * 128, :], ev[:])
                    # per-token sumsq via fused square + accumulate
                    sqt = epool.tile([128, 512], F32R, tag="sq")
                    nc.vector.scalar_tensor_tensor(
                        sqt[:], ev[:], 1.0, ev[:], op0=ALU.mult, op1=ALU.mult,
                        accum_out=vss[:, n * 4 + mt:n * 4 + mt + 1])
                # g (T-layout): store silu(g) directly
                for m in range(4):
                    ps = ps_big()
                    tproj_mms(ps, wg_sb, xth, m)
                    ev = epool.tile([128, 512], F32R, tag="ev")
                    nc.scalar.activation(ev[:], ps[:], ACTF.Silu)
                    nc.sync.dma_start(gT_s[m * 128:(m + 1) * 128, tok], ev[:])
                # gt logits: accumulate x and c streams
                gtps = ps_small([128, 512])
                for k in range(16):
                    nc.tensor.matmul(gtps[:HPC, :], wgt_sb[:, k, :],
                                     xth[k // 8][:, k % 8, :],
                                     start=(k == 0), stop=False)
                for k in range(16):
                    ct = ctpool.tile([128, 512], F32R, tag="ct", bufs=8)
                    nc.sync.dma_start(ct[:], cT[k * 128:(k + 1) * 128, tok])
                    nc.tensor.matmul(gtps[:HPC, :], wgt_sb[:, k, :], ct[:],
                                     start=False, stop=(k == 15))
                gstg = spool.tile([HPC, 512], F32, tag="gstg", bufs=1)
                nc.vector.tensor_copy(gstg[:], gtps[:HPC, :])
                for j in range(4):
                    tp = ps_small([128, HPC])
                    nc.tensor.matmul(tp[:], gstg[:, j * 128:(j + 1) * 128],
                                     ident32[:HPC, :HPC], is_transpose=True)
                    nc.vector.tensor_copy(gtn[:, n * 4 + j, :], tp[:])

            # v sumsq: transpose [128, 32] -> [32, 128] -> ss_in row 2
            vssT = ps_small([128, 128])
            nc.tensor.matmul(vssT[:32, :], vss[:], ident32,
                             is_transpose=True)
            vssev = spool.tile([32, 128], F32, tag="vssev")
            nc.vector.tensor_copy(vssev[:], vssT[:32, :])
            nc.sync.dma_start(
                ss_in[2:3, :].rearrange("a (b c) -> (a b) c", c=128), vssev[:])

            # =========================================================
            # P2: AllReduce sumsq; scales; gate decays
            # =========================================================
            nc.gpsimd.collective_compute(
                "AllReduce", ALU.add,
                replica_groups=[[0, 1, 2, 3], [4, 5, 6, 7]],
                ins=[ss_in[:].opt()], outs=[ss_out[:].opt()],
            )
            ssn = ppool.tile([128, NBLK, 3], F32, tag="ssn")
            for nn_ in range(NT):
                tok = slice(nn_ * 512, (nn_ + 1) * 512)
                srt = spool.tile([3, 512], F32, tag="srt", bufs=1)
                nc.sync.dma_start(srt[:], ss_out[:, tok])
                for j in range(4):
                    tp = ps_small([128, 4])
                    nc.tensor.matmul(tp[:, :3], srt[:, j * 128:(j + 1) * 128],
                                     ident32[:3, :3], is_transpose=True)
                    nc.vector.tensor_copy(ssn[:, nn_ * 4 + j, :], tp[:, :3])
            rsn = ppool.tile([128, NBLK, 3], F32, tag="rsn")
            nc.vector.tensor_scalar(rsn[:], ssn[:], 1.0 / DIM, EPS,
                                    ALU.mult, ALU.add)
            nc.scalar.activation(rsn[:], rsn[:], ACTF.Ln)
            nc.scalar.activation(rsn[:], rsn[:], ACTF.Exp, scale=-0.5)
            skv = ppool.tile([128, NBLK], F32, tag="skv")
            nc.vector.tensor_mul(skv[:], rsn[:, :, 1], rsn[:, :, 2])
            if debug:
                nc.sync.dma_start(dbg("dbg_rsn", [128, NBLK * 3]),
                                  rsn[:].rearrange("p a b -> p (a b)"))

            # gate decays: sp = softplus(-z) = ln(1 + exp(-z)); -1/GLN in Lm/Om
            gtd = ppool.tile([128, NBLK, HPC], F32R, tag="gtd")
            nc.scalar.activation(gtn[:], gtn[:], ACTF.Exp, scale=-1.0)
            nc.scalar.activation(gtd[:], gtn[:], ACTF.Ln, bias=1.0)

            # per chunk: b (2 blocks) and tn via triangular matmuls; factors
            rf = ppool.tile([128, NCH, 2, HPC], F32, tag="rf")      # rowfac
            vf = ppool.tile([128, NCH, 2, HPC], F32, tag="vf")      # vfac
            etn = ppool.tile([128, NCH, HPC], F32, tag="etn")       # exp(tn)
            for ch in range(NCH):
                b0, b1 = 2 * ch, 2 * ch + 1
                p0 = ps_small([128, HPC])
                nc.tensor.matmul(p0[:], Lm, gtd[:, b0, :], start=True, stop=True)
                p1 = ps_small([128, HPC])
                nc.tensor.matmul(p1[:], Om, gtd[:, b0, :], start=True, stop=False)
                nc.tensor.matmul(p1[:], Lm, gtd[:, b1, :], start=False, stop=True)
                pt = ps_small([128, HPC])
                nc.tensor.matmul(pt[:], Om, gtd[:, b0, :], start=True, stop=False)
                nc.tensor.matmul(pt[:], Om, gtd[:, b1, :], start=False, stop=True)
                nc.scalar.activation(etn[:, ch, :], pt[:], ACTF.Exp)
                for blk01, bps in ((0, p0), (1, p1)):
                    blk = 2 * ch + blk01
                    # rowfac = exp(b) * sq * scale
                    nc.scalar.activation(rf[:, ch, blk01, :], bps[:], ACTF.Exp)
                    nc.vector.tensor_scalar(
                        rf[:, ch, blk01, :], rf[:, ch, blk01, :],
                        rsn[:, blk, 0:1], SCALE, ALU.mult, ALU.mult)
                    # vfac = exp(-b) * sk * sv
                    nc.scalar.activation(vf[:, ch, blk01, :], bps[:], ACTF.Exp,
                                         scale=-1.0)
                    nc.vector.tensor_scalar(
                        vf[:, ch, blk01, :], vf[:, ch, blk01, :],
                        skv[:, blk:blk + 1], None, ALU.mult)

            # =========================================================
            # P3: retention + gating + out-proj, per chunk
            # =========================================================
            wo_sb = wpool.tile([128, HPC, DIM], F32R, tag="w0")
            nc.sync.dma_start(wo_sb[:], wo.rearrange("(h p) m -> p h m", p=128))

            S_prev = [None] * HPC
            for ch in range(NCH):
                tok = slice(ch * CS, (ch + 1) * CS)
                qc = rpool.tile([128, HPC, CS], F32R, tag="qc")
                kc = rpool.tile([128, HPC, CS], F32R, tag="kc")
                for t_, s_ in ((qc, qT_s), (kc, kT_s)):
                    nc.sync.dma_start(
                        t_[:], s_[:, tok].rearrange("(h p) m -> p h m", p=128))
                vcn, sg = [], []
                for blk01 in range(2):
                    bt = slice(ch * CS + blk01 * 128, ch * CS + blk01 * 128 + 128)
                    vt = rpool.tile([128, PCOLS], F32R, tag="vcn")
                    nc.sync.dma_start(vt[:], vN_s[bt, :])
                    vcn.append(vt)
                    gt_ = rpool.tile([128, HPC, 128], F32R, tag="gch")
                    nc.sync.dma_start(
                        gt_[:], gT_s[:, bt].rearrange("(h p) m -> p h m", p=128))
                    sg.append(gt_)
                o_st = rpool.tile([128, 2 * HPC, HD], F32, tag="o_st")
                for hl in range(HPC):
                    # k_nat via PE transpose; vv from natural v
                    knat, vvt = [], []
                    for blk01 in range(2):
                        bsl = slice(blk01 * 128, blk01 * 128 + 128)
                        tpk = ps_small([128, 128], F32R)
                        nc.tensor.transpose(tpk[:], kc[:, hl, bsl], ident)
                        kn = rpool.tile([128, 128], F32R, tag="knat")
                        nc.scalar.copy(kn[:], tpk[:])
                        knat.append(kn)
                        vv = rpool.tile([128, 128], F32R, tag="vv")
                        nc.gpsimd.tensor_scalar(
                            vv[:], vcn[blk01][:, hl * 128:(hl + 1) * 128],
                            vf[:, ch, blk01, hl:hl + 1], None, ALU.mult)
                        vvt.append(vv)
                    # AT (masked): rows cj, cols ci
                    at0ps = ps_small([128, 256])
                    nc.tensor.matmul(at0ps[:], kc[:, hl, 0:128], qc[:, hl, :],
                                     start=True, stop=True)
                    at0 = rpool.tile([128, CS], F32R, tag="at0")
                    nc.vector.scalar_tensor_tensor(
                        at0[:, 0:128], at0ps[:, 0:128], 1.0, Um32,
                        op0=ALU.mult, op1=ALU.mult)
                    nc.scalar.copy(at0[:, 128:256], at0ps[:, 128:256])
                    at1ps = ps_small([128, 128])
                    nc.tensor.matmul(at1ps[:], kc[:, hl, 128:256],
                                     qc[:, hl, 128:256], start=True, stop=True)
                    at1 = rpool.tile([128, 128], F32R, tag="at1s")
                    nc.vector.scalar_tensor_tensor(
                        at1[:], at1ps[:], 1.0, Um32, op0=ALU.mult, op1=ALU.mult)
                    # o = intra + inter (one PSUM group per ci half)
                    for ci in range(2):
                        csl = slice(ci * 128, ci * 128 + 128)
                        mms = [(at0[:, csl], vvt[0][:])]
                        if ci == 1:
                            mms.append((at1[:], vvt[1][:]))
                        if ch > 0:
                            mms.append((qc[:, hl, csl], S_prev[hl][:]))
                        ops = ps_small([128, HD])
                        for i, (lh, rh) in enumerate(mms):
                            nc.tensor.matmul(ops[:], lh, rh, start=(i == 0),
                                             stop=(i == len(mms) - 1))
                        nc.scalar.mul(o_st[:, ci * HPC + hl, :], ops[:],
                                      rf[:, ch, ci, hl:hl + 1])
                    # state update: S_cur = (S_prev + contrib) * exp(tn)
                    sps = ps_small([128, HD])
                    nc.tensor.matmul(sps[:], knat[0][:], vvt[0][:],
                                     start=True, stop=False)
                    nc.tensor.matmul(sps[:], knat[1][:], vvt[1][:],
                                     start=False, stop=True)
                    S_cur = rpool.tile([128, HD], F32R, tag=f"S{hl}")
                    if ch > 0:
                        stmp = rpool.tile([128, HD], F32, tag="stmp")
                        nc.vector.tensor_add(stmp[:], S_prev[hl][:].bitcast(F32),
                                             sps[:])
                        nc.gpsimd.tensor_scalar(
                            S_cur[:], stmp[:], etn[:, ch, hl:hl + 1], None,
                            ALU.mult)
                    else:
                        nc.vector.tensor_scalar(
                            S_cur[:], sps[:], etn[:, ch, hl:hl + 1], None,
                            ALU.mult)
                    S_prev[hl] = S_cur
                # o-norm over head dim (free)
                osq = rpool.tile([128, 2 * HPC, HD], F32, tag="osq", bufs=1)
                nc.scalar.activation(osq[:], o_st[:], ACTF.Square)
                ssum = rpool.tile([128, 2 * HPC], F32, tag="ossum")
                nc.vector.tensor_reduce(ssum[:], osq[:], AX.X, ALU.add)
                nc.vector.tensor_scalar(ssum[:], ssum[:], 1.0 / HD, EPS,
                                        ALU.mult, ALU.add)
                nc.vector.reciprocal(ssum[:], ssum[:])
                nc.scalar.activation(ssum[:], ssum[:], ACTF.Sqrt)
                o_n = rpool.tile([128, 2 * HPC, HD], F32R, tag="o_n", bufs=1)
                nc.vector.tensor_tensor(
                    o_n[:], o_st[:],
                    ssum[:].unsqueeze(2).to_broadcast([128, 2 * HPC, HD]),
                    ALU.mult)
                # transpose + gate into go_st
                go_st = rpool.tile([128, HPC, CS], F32R, tag="go_st")
                for hl in range(HPC):
                    for blk01 in range(2):
                        trp = ps_small([128, 128], F32R)
                        nc.tensor.transpose(
                            trp[:], o_n[:][:, blk01 * HPC + hl, :], ident)
                        bsl = slice(blk01 * 128, blk01 * 128 + 128)
                        nc.vector.tensor_mul(
                            go_st[:, hl, bsl], trp[:],
                            sg[blk01][:, hl, :])
                # out-proj for this chunk's two token tiles
                for m01 in range(2):
                    msl = slice(m01 * 128, m01 * 128 + 128)
                    for n in range(DIM // 512):
                        ps = ps_big()
                        nsl = slice(n * 512, (n + 1) * 512)
                        for k in range(HPC):
                            nc.tensor.matmul(ps[:], go_st[:, k, msl],
                                             wo_sb[:, k, nsl],
                                             start=(k == 0), stop=(k == HPC - 1))
                        oo = epool.tile([128, 512], F32, tag="oo", bufs=2)
                        if n % 2 == 0:
                            nc.vector.tensor_copy(oo[:], ps[:])
                        else:
                            nc.scalar.copy(oo[:], ps[:])
                        nc.sync.dma_start(
                            out[ch * CS + m01 * 128:ch * CS + m01 * 128 + 128,
                                nsl], oo[:])

    nc.compile()
    return nc


def _prep_inputs(x, c, Wq, Wk, Wv, Wg, Wgt, Wo):
    """Build the 8 per-core input maps (host-side sharding / layout)."""
    consts = np.ascontiguousarray(_consts_np())
    in_maps = []
    xTs = [np.ascontiguousarray(x[b].T) for b in range(B)]
    cTs = [np.ascontiguousarray(c[b].T) for b in range(B)]
    for core in range(NCORE):
        b, g = core // 4, core % 4
        cols = slice(g * PCOLS, (g + 1) * PCOLS)
        heads = slice(g * HPC, (g + 1) * HPC)
        in_maps.append({
            "xT": xTs[b],
            "cT": cTs[b],
            "wq": np.ascontiguousarray(Wq[:, cols]),
            "wk": np.ascontiguousarray(Wk[:, cols]),
            "wv": np.ascontiguousarray(Wv[:, cols]),
            "wg": np.ascontiguousarray(Wg[:, cols]),
            "wgt": np.ascontiguousarray(Wgt[:, heads]),
            "wo": np.ascontiguousarray(Wo[cols, :]),
            "consts": consts,
        })
    return in_maps


def kernel(x, c, Wq, Wk, Wv, Wg, Wgt, Wo, _want_results=False):
    key = "nc_dbg" if DEBUG else "nc"
    if key not in _cache:
        _cache[key] = build(debug=DEBUG)
    nc = _cache[key]
    in_maps = _prep_inputs(np.asarray(x, np.float32), np.asarray(c, np.float32),
                           np.asarray(Wq, np.float32), np.asarray(Wk, np.float32),
                           np.asarray(Wv, np.float32), np.asarray(Wg, np.float32),
                           np.asarray(Wgt, np.float32), np.asarray(Wo, np.float32))
    res = bass_utils.run_bass_kernel_spmd(
        nc, in_maps, core_ids=list(range(NCORE)), trace=TRACE)
    out = np.zeros((B, T, DIM), np.float32)
    for core in range(NCORE):
        out[core // 4] += res.results[core]["out"]
    if _want_results:
        return out, res
    return out
